# revision 62
# baseline (speedup 1.0000x reference)
"""Trainium2 Bass kernel for nn_DecoderTransformerBackbone_1589137900084.

Decoder transformer backbone: B=8, N=2048, D=256, L=4 layers of
relu-attention with a causal averaging mask + MLP, layernorms after each
residual. Data-parallel over batch: one batch element per NeuronCore (8 cores).

Per-core layout strategy (v2, restructured from the 335us baseline):
  - H (the residual stream) lives in SBUF as 16 tiles of [128 tokens, 256
    dims], dtype f32r (PE transposes run 1.5 cyc/row instead of f32's 2).
  - One shared PSUM pool of 8 rotating [128,512] single-bank slots feeds
    every phase (transposes, QKV, S, AV, MLP), maximizing the number of
    matmul outputs in flight so the PE never waits on PSUM drain copies.
  - Emission is interleaved so every PE instruction's inputs were produced
    several instructions earlier: T1 g0..g3 -> QK cp0 -> V p0,p1 -> QK cp1 ->
    S/AV ic0 -> V p2,p3 -> QK cp2 -> S/AV ic1 -> ... etc.
  - DMAs are ordered h0-first then layer-0 weights so the PE starts ~25us
    earlier than the load-everything-then-run baseline.
  - The in-block causal triangle multiply runs on the otherwise-idle GPSIMD
    engine; PSUM->SBUF drains alternate between Activation and DVE.
  - LN group stats: Sqrt activation (bias=eps, scale=1/D) + DVE reciprocal
    (bass blocks the Rsqrt table for accuracy reasons).
"""
import sys

sys.path.insert(0, "/opt/trn_rl_repo")

import numpy as np

B, N, D, L = 8, 2048, 256, 4
LN_EPS = 1e-5
P = 128
NT = N // P            # 16 token tiles
DT = D // P            # 2 dim tiles
IC = N // 512          # 4 free-dim chunks of 512

_CACHE = {}
_last_in_maps = None
REPEAT = 1
PROFILE = False
LAST_EXEC_NS = None
LAST_RESULTS = None


def _build_program(use_b1, use_b2, use_ln1_gb, use_ln2_gb):
    import concourse.bass as bass  # noqa: F401
    from concourse import bacc
    import concourse.mybir as mybir
    import concourse.tile as tile

    f32 = mybir.dt.float32
    f32r = mybir.dt.float32r
    bf16 = mybir.dt.bfloat16
    AF = mybir.ActivationFunctionType
    OP = mybir.AluOpType

    nc = bacc.Bacc("TRN2", target_bir_lowering=False)

    h0_d = nc.declare_dram_parameter("h0", [N, D], bf16, isOutput=False)
    wq_d = nc.declare_dram_parameter("wq", [L, D, D], bf16, isOutput=False)
    wk_d = nc.declare_dram_parameter("wk", [L, D, D], bf16, isOutput=False)
    wv_d = nc.declare_dram_parameter("wv", [L, D, D], bf16, isOutput=False)
    w1_d = nc.declare_dram_parameter("w1", [L, D, D], bf16, isOutput=False)
    w2_d = nc.declare_dram_parameter("w2", [L, D, D], f32r, isOutput=False)
    tri_d = nc.declare_dram_parameter("tri", [P, P], f32r, isOutput=False)
    invpos_d = nc.declare_dram_parameter("invpos", [P, NT], f32, isOutput=False)
    if use_b1:
        b1_d = nc.declare_dram_parameter("b1", [L, D], f32, isOutput=False)
    if use_b2:
        b2_d = nc.declare_dram_parameter("b2", [L, D], f32, isOutput=False)
    if use_ln1_gb:
        ln1g_d = nc.declare_dram_parameter("ln1g", [L, D], f32, isOutput=False)
        ln1b_d = nc.declare_dram_parameter("ln1b", [L, D], f32, isOutput=False)
    if use_ln2_gb:
        ln2g_d = nc.declare_dram_parameter("ln2g", [L, D], f32, isOutput=False)
        ln2b_d = nc.declare_dram_parameter("ln2b", [L, D], f32, isOutput=False)
    out_d = nc.declare_dram_parameter("out", [N, D], bf16, isOutput=True)

    with tile.TileContext(nc) as tc:
        with (
            tc.tile_pool(name="const", bufs=1) as constp,
            tc.tile_pool(name="work", bufs=1) as workp,
            tc.tile_pool(name="stp", bufs=20) as stp,
            tc.tile_pool(name="sqp", bufs=3) as sqp,
            tc.tile_pool(name="small", bufs=8) as smallp,
            tc.tile_pool(name="ps", bufs=8, space="PSUM") as psp,
        ):
            # ---------------- constants & activations (DMA order matters:
            # ident+h0 first so transposes start immediately, then layer-0
            # weights, then the rest) ----------------
            # build the transpose identity on-chip (no DMA on the critical
            # path): keep zeros where p != j, fill 1.0 on the diagonal
            ident = constp.tile([P, P], bf16, tag="ident")
            nc.vector.memset(ident[:], 0.0)
            nc.gpsimd.affine_select(
                out=ident[:], in_=ident[:], pattern=[[-1, P]],
                compare_op=OP.not_equal, fill=1.0,
                base=0, channel_multiplier=1)

            H = [workp.tile([P, D], bf16, tag=f"h_{nt}", name=f"h_{nt}")
                 for nt in range(NT)]

            def load_h0_group(g):
                # tiles 4g..4g+3 in one DMA each is impossible (separate
                # tiles), but 4 per-tile DMAs batch poorly on HWDGE; instead
                # two tiles per DMA via their adjacent pool slots is fragile,
                # so keep per-tile DMAs but emit them group-wise so weight
                # loads interleave between groups.
                for k in range(4):
                    nt = 4 * g + k
                    nc.sync.dma_start(H[nt][:], h0_d[nt * P:(nt + 1) * P, :])

            W = {}

            def load_w(wname, wd, l):
                dtype = f32r if wname == "w2" else bf16
                for dt_ in range(DT):
                    t = constp.tile([P, D], dtype, tag=f"{wname}_{l}_{dt_}")
                    nc.sync.dma_start(t[:], wd[l, dt_ * P:(dt_ + 1) * P, :])
                    W[wname, l, dt_] = t

            tri = constp.tile([P, P], f32r, tag="tri")
            invpos = constp.tile([P, NT], f32, tag="invpos")
            eps_t = constp.tile([P, 1], f32, tag="eps")

            load_h0_group(0)
            load_w("wq", wq_d, 0)
            load_w("wk", wk_d, 0)
            load_h0_group(1)
            load_w("wv", wv_d, 0)
            load_h0_group(2)
            load_w("w1", w1_d, 0)
            load_h0_group(3)
            load_w("w2", w2_d, 0)
            nc.sync.dma_start(tri[:], tri_d[:])
            nc.sync.dma_start(invpos[:], invpos_d[:])
            for l in range(1, L):
                for wname, wd in (("wq", wq_d), ("wk", wk_d), ("wv", wv_d),
                                  ("w1", w1_d), ("w2", w2_d)):
                    load_w(wname, wd, l)
            nc.vector.memset(eps_t[:], LN_EPS)

            def load_vec(dram, tag):
                out = []
                for l in range(L):
                    t = constp.tile([P, DT], f32, tag=f"{tag}_{l}")
                    nc.sync.dma_start(
                        t[:], dram[l].rearrange("(dt p) -> p dt", p=P))
                    out.append(t)
                return out

            def load_bcast(dram, tag):
                out = []
                for l in range(L):
                    t = constp.tile([P, D], f32, tag=f"{tag}b_{l}")
                    nc.sync.dma_start(
                        t[:], dram[l].unsqueeze(0).to_broadcast([P, D]))
                    out.append(t)
                return out

            b1_t = load_vec(b1_d, "b1") if use_b1 else None
            b2_t = load_bcast(b2_d, "b2") if use_b2 else None
            ln1g_t = load_bcast(ln1g_d, "ln1g") if use_ln1_gb else None
            ln1b_t = load_bcast(ln1b_d, "ln1b") if use_ln1_gb else None
            ln2g_t = load_bcast(ln2g_d, "ln2g") if use_ln2_gb else None
            ln2b_t = load_bcast(ln2b_d, "ln2b") if use_ln2_gb else None

            qT = [workp.tile([P, N], bf16, tag=f"qt_{d}", name=f"qt_{d}")
                  for d in range(DT)]
            kT = [workp.tile([P, N], bf16, tag=f"kt_{d}", name=f"kt_{d}")
                  for d in range(DT)]
            h1T = [workp.tile([P, N], f32r, tag=f"h1t_{d}", name=f"h1t_{d}")
                   for d in range(DT)]
            HT = [[workp.tile([P, N], bf16, tag=f"ht_{d}", name=f"ht_{d}")
                   for d in range(DT)]]
            Vp = [[workp.tile([P, 2 * D], f32r, tag=f"vp_{i}", name=f"vp_{i}")
                   for i in range(NT // 2)]]

            def Vsl(nt):
                return Vp[0][nt // 2][:, (nt % 2) * D:(nt % 2 + 1) * D]

            def ps_slot(name, dtype=f32):
                return psp.tile([P, 512], dtype, tag="ps", name=name)

            _alt = [0]

            def psum_copy(dst, src, relu=False):
                # alternate Act / DVE for PSUM->SBUF drains
                _alt[0] ^= 1
                if _alt[0]:
                    nc.scalar.activation(dst, src, AF.Relu if relu else AF.Copy)
                elif relu:
                    nc.vector.tensor_scalar_max(dst, src, 0.0)
                else:
                    nc.vector.tensor_copy(dst, src)

            # ---------------- per-phase emitters ----------------
            def emit_T_pair(pair, dst):
                # transpose H tiles 2*pair..2*pair+1 into dst cols pair*256..
                for dt_ in range(DT):
                    ps = ps_slot("ps_t", bf16)
                    for k in range(2):
                        nt = 2 * pair + k
                        nc.tensor.transpose(
                            ps[:, k * P:(k + 1) * P],
                            H[nt][:, dt_ * P:(dt_ + 1) * P],
                            ident[:],
                        )
                    psum_copy(dst[0][dt_][:, pair * 256:(pair + 1) * 256],
                              ps[:, :256])

            def emit_h1_half(hh, l):
                # 256-wide h1 chunk consuming T2 pair hh (cols hh*256..)
                ps = ps_slot("ps_h1h")
                for et in range(DT):
                    for dt_ in range(DT):
                        nc.tensor.matmul(
                            ps[:, et * 256:(et + 1) * 256],
                            W["w1", l, dt_][:, et * P:(et + 1) * P],
                            HT[0][dt_][:, hh * 256:(hh + 1) * 256],
                            start=(dt_ == 0), stop=(dt_ == DT - 1),
                            skip_group_check=True,
                        )
                for et in range(DT):
                    dst = h1T[et][:, hh * 256:(hh + 1) * 256]
                    if use_b1:
                        nc.scalar.activation(dst, ps[:, et * 256:(et + 1) * 256],
                                             AF.Relu, bias=b1_t[l][:, et:et + 1])
                    elif et == 0:
                        nc.scalar.activation(dst, ps[:, et * 256:(et + 1) * 256],
                                             AF.Relu)
                    else:
                        nc.vector.tensor_scalar_max(
                            dst, ps[:, et * 256:(et + 1) * 256], 0.0)

            def emit_qk_half(half, wname, dst, l):
                # 256-wide first-chunk qk consuming one transpose pair;
                # both et halves share one psum slot
                ps = ps_slot("ps_qk")
                for et in range(DT):
                    for dt_ in range(DT):
                        nc.tensor.matmul(
                            ps[:, et * 256:(et + 1) * 256],
                            W[wname, l, dt_][:, et * P:(et + 1) * P],
                            HT[0][dt_][:, half * 256:(half + 1) * 256],
                            start=(dt_ == 0), stop=(dt_ == DT - 1),
                            skip_group_check=True,
                        )
                for et in range(DT):
                    psum_copy(dst[et][:, half * 256:(half + 1) * 256],
                              ps[:, et * 256:(et + 1) * 256])

            def emit_T(g, dst):
                # transpose H tiles 4g..4g+3 into dst[:, g*512:(g+1)*512]
                for dt_ in range(DT):
                    ps = ps_slot("ps_t", bf16)
                    for k in range(4):
                        nt = 4 * g + k
                        nc.tensor.transpose(
                            ps[:, k * P:(k + 1) * P],
                            H[nt][:, dt_ * P:(dt_ + 1) * P],
                            ident[:],
                        )
                    psum_copy(dst[0][dt_][:, g * 512:(g + 1) * 512], ps[:])

            def emit_qk(cp, wname, dst, l):
                # produce dst[et][:, cp*512:(cp+1)*512] for et in 0,1
                for et in range(DT):
                    ps = ps_slot("ps_qk")
                    for dt_ in range(DT):
                        nc.tensor.matmul(
                            ps[:],
                            W[wname, l, dt_][:, et * P:(et + 1) * P],
                            HT[0][dt_][:, cp * 512:(cp + 1) * 512],
                            start=(dt_ == 0), stop=(dt_ == DT - 1),
                            skip_group_check=True,
                        )
                    psum_copy(dst[et][:, cp * 512:(cp + 1) * 512], ps[:])

            def emit_v(pair, l):
                ps = ps_slot("ps_v")
                for k in range(2):
                    nt = 2 * pair + k
                    for dt_ in range(DT):
                        nc.tensor.matmul(
                            ps[:, k * D:(k + 1) * D],
                            HT[0][dt_][:, nt * P:(nt + 1) * P],
                            W["wv", l, dt_][:],
                            start=(dt_ == 0), stop=(dt_ == DT - 1),
                            skip_group_check=True,
                        )
                psum_copy(Vp[0][pair][:], ps[:])

            def ln_group_stats(sum_g, ssq_g, n, tag):
                # rstd = rsqrt((ssq - sum^2/D)/D + eps); nmu = -sum/D*rstd
                sq = smallp.tile([P, 4], f32, tag=f"lnsq{tag}", name="lnsq")
                rstd = smallp.tile([P, 4], f32, tag=f"lnrstd{tag}", name="lnrstd")
                nmu = smallp.tile([P, 4], f32, tag=f"lnnmu{tag}", name="lnnmu")
                nc.vector.tensor_tensor(out=sq[:, :n], in0=sum_g[:, :n],
                                        in1=sum_g[:, :n], op=OP.mult)
                nc.vector.scalar_tensor_tensor(
                    out=sq[:, :n], in0=sq[:, :n], scalar=-1.0 / D,
                    in1=ssq_g[:, :n], op0=OP.mult, op1=OP.add)
                nc.scalar.activation(rstd[:, :n], sq[:, :n], AF.Sqrt,
                                     bias=eps_t[:], scale=1.0 / D)
                nc.vector.reciprocal(rstd[:, :n], rstd[:, :n])
                nc.vector.scalar_tensor_tensor(
                    out=nmu[:, :n], in0=sum_g[:, :n], scalar=-1.0 / D,
                    in1=rstd[:, :n], op0=OP.mult, op1=OP.mult)
                return rstd, nmu

            def ln_sumsq(h, dst, k):
                if k % 2 == 0:
                    sq = sqp.tile([P, D], bf16, tag="sqs", name="sqs")
                    nc.scalar.activation(sq[:], h[:], AF.Square, accum_out=dst)
                else:
                    sq = sqp.tile([P, D], bf16, tag="sqv", name="sqv")
                    nc.vector.scalar_tensor_tensor(
                        out=sq[:], in0=h[:], scalar=1.0, in1=h[:],
                        op0=OP.mult, op1=OP.mult, accum_out=dst)

            def ln_apply(h, rstd, nmu, k, g_t, b_t):
                if k % 2 == 0:
                    nc.scalar.activation(h[:], h[:], AF.Identity,
                                         scale=rstd[:, k:k + 1],
                                         bias=nmu[:, k:k + 1])
                else:
                    nc.vector.tensor_scalar(
                        out=h[:], in0=h[:], scalar1=rstd[:, k:k + 1],
                        scalar2=nmu[:, k:k + 1], op0=OP.mult, op1=OP.add)
                if g_t is not None:
                    nc.vector.tensor_tensor(out=h[:], in0=h[:], in1=g_t[:],
                                            op=OP.mult)
                    nc.vector.tensor_tensor(out=h[:], in0=h[:], in1=b_t[:],
                                            op=OP.add)

            def emit_s_block(ic, jt, STl):
                c0 = P * max(0, jt - 4 * ic)
                # qT/kT are bf16 (1 cyc/row at ANY width), so diagonal blocks
                # can run at their exact width - no fp32r >=256 clamp
                c0p = c0
                ps = ps_slot("ps_s")
                for et in range(DT):
                    nc.tensor.matmul(
                        ps[:, c0p:],
                        kT[et][:, jt * P:(jt + 1) * P],
                        qT[et][:, ic * 512 + c0p:(ic + 1) * 512],
                        start=(et == 0), stop=(et == DT - 1),
                    )
                st = stp.tile([P, 512], f32r, tag="st", name="st")
                psum_copy(st[:, c0:], ps[:, c0:], relu=True)
                if jt >= 4 * ic:
                    nc.gpsimd.tensor_tensor(
                        out=st[:, c0:c0 + P], in0=st[:, c0:c0 + P],
                        in1=tri[:], op=OP.mult)
                STl.append(st)

            def emit_av(ib, STl, sum_g, ib_l):
                ps = ps_slot("ps_av")
                for jt in range(ib + 1):
                    nc.tensor.matmul(
                        ps[:, :D],
                        STl[jt][:, ib_l * P:(ib_l + 1) * P],
                        Vsl(jt),
                        start=(jt == 0), stop=(jt == ib),
                    )
                nc.vector.scalar_tensor_tensor(
                    out=H[ib][:], in0=ps[:, :D],
                    scalar=invpos[:, ib:ib + 1], in1=H[ib][:],
                    op0=OP.mult, op1=OP.add,
                    accum_out=sum_g[:, ib_l:ib_l + 1])

            def emit_h1(cp, l):
                for et in range(DT):
                    ps = ps_slot("ps_h1")
                    for dt_ in range(DT):
                        nc.tensor.matmul(
                            ps[:],
                            W["w1", l, dt_][:, et * P:(et + 1) * P],
                            HT[0][dt_][:, cp * 512:(cp + 1) * 512],
                            start=(dt_ == 0), stop=(dt_ == DT - 1),
                            skip_group_check=True,
                        )
                    dst = h1T[et][:, cp * 512:(cp + 1) * 512]
                    _alt[0] ^= 1
                    if use_b1:
                        nc.scalar.activation(dst, ps[:], AF.Relu,
                                             bias=b1_t[l][:, et:et + 1])
                    elif _alt[0]:
                        nc.scalar.activation(dst, ps[:], AF.Relu)
                    else:
                        nc.vector.tensor_scalar_max(dst, ps[:], 0.0)

            def emit_m2(nt, l, sum_g, k):
                ps = ps_slot("ps_m2")
                for et in range(DT):
                    nc.tensor.matmul(
                        ps[:, :D],
                        h1T[et][:, nt * P:(nt + 1) * P],
                        W["w2", l, et][:],
                        start=(et == 0), stop=(et == DT - 1),
                    )
                if use_b2:
                    nc.vector.scalar_tensor_tensor(
                        out=H[nt][:], in0=ps[:, :D], scalar=1.0,
                        in1=H[nt][:], op0=OP.mult, op1=OP.add)
                    nc.vector.tensor_tensor(out=H[nt][:], in0=H[nt][:],
                                            in1=b2_t[l][:], op=OP.add)
                    nc.vector.tensor_reduce(
                        out=sum_g[:, k:k + 1], in_=H[nt][:],
                        axis=mybir.AxisListType.X, op=OP.add)
                else:
                    nc.vector.scalar_tensor_tensor(
                        out=H[nt][:], in0=ps[:, :D], scalar=1.0,
                        in1=H[nt][:], op0=OP.mult, op1=OP.add,
                        accum_out=sum_g[:, k:k + 1])

            # ---------------- main layer loop ----------------
            for li in range(L * REPEAT):
                l = li % L

                sum1 = smallp.tile([P, NT], f32, tag="sum1", name="sum1")
                ssq1 = smallp.tile([P, NT], f32, tag="ssq1", name="ssq1")
                sum2 = smallp.tile([P, NT], f32, tag="sum2", name="sum2")
                ssq2 = smallp.tile([P, NT], f32, tag="ssq2", name="ssq2")

                def emit_s_chunk(ic):
                    STl = []
                    for jt in range(4 * ic + 4):
                        emit_s_block(ic, jt, STl)
                    return STl

                def emit_av_ln(ic, STl):
                    for ib_l in range(4):
                        ib = 4 * ic + ib_l
                        emit_av(ib, STl, sum1[:, 4 * ic:4 * ic + 4], ib_l)
                        ln_sumsq(H[ib], ssq1[:, ib:ib + 1], ib_l)
                    rstd, nmu = ln_group_stats(
                        sum1[:, 4 * ic:4 * ic + 4], ssq1[:, 4 * ic:4 * ic + 4],
                        4, "1")
                    for ib_l in range(4):
                        ib = 4 * ic + ib_l
                        ln_apply(H[ib], rstd, nmu, ib_l,
                                 ln1g_t[l] if use_ln1_gb else None,
                                 ln1b_t[l] if use_ln1_gb else None)

                def mlp_chunk(g):
                    for k in range(4):
                        nt = 4 * g + k
                        emit_m2(nt, l, sum2[:, 4 * g:4 * g + 4], k)
                        ln_sumsq(H[nt], ssq2[:, nt:nt + 1], k)
                    rstd, nmu = ln_group_stats(
                        sum2[:, 4 * g:4 * g + 4], ssq2[:, 4 * g:4 * g + 4],
                        4, "2")
                    for k in range(4):
                        nt = 4 * g + k
                        ln_apply(H[nt], rstd, nmu, k,
                                 ln2g_t[l] if use_ln2_gb else None,
                                 ln2b_t[l] if use_ln2_gb else None)
                        if li == L * REPEAT - 1:
                            nc.sync.dma_start(
                                out_d[nt * P:(nt + 1) * P, :], H[nt][:])

                # --- phase A: transposes + QKV interleaved with S/AV ---
                # pair-granular start: qk(cp0) halves consume transpose pairs
                # as soon as their two LN applies land
                emit_T_pair(0, HT)
                emit_qk_half(0, "wq", qT, l)
                emit_T_pair(1, HT)
                emit_qk_half(0, "wk", kT, l)
                emit_qk_half(1, "wq", qT, l)
                emit_qk_half(1, "wk", kT, l)
                emit_v(0, l)
                emit_T(1, HT)
                emit_v(1, l)
                st0 = emit_s_chunk(0)
                emit_T(2, HT)
                emit_qk(1, "wq", qT, l)
                emit_qk(1, "wk", kT, l)
                emit_T(3, HT)
                emit_av_ln(0, st0)
                emit_v(2, l)
                emit_v(3, l)
                st1 = emit_s_chunk(1)
                emit_qk(2, "wq", qT, l)
                emit_qk(2, "wk", kT, l)
                emit_av_ln(1, st1)
                emit_v(4, l)
                emit_v(5, l)
                st2 = emit_s_chunk(2)
                emit_qk(3, "wq", qT, l)
                emit_qk(3, "wk", kT, l)
                emit_v(6, l)
                emit_v(7, l)
                emit_av_ln(2, st2)
                # --- phase B: MLP (T2/h1 interleave into the attn tail) ---
                st3 = emit_s_chunk(3)
                emit_T(0, HT)
                emit_av_ln(3, st3)
                emit_h1(0, l)
                emit_T(1, HT)
                emit_h1(1, l)
                emit_T(2, HT)
                mlp_chunk(0)
                emit_h1(2, l)
                emit_T(3, HT)
                mlp_chunk(1)
                emit_h1(3, l)
                mlp_chunk(2)
                mlp_chunk(3)

    nc.finalize()
    return nc


def kernel(**inputs):
    global LAST_EXEC_NS, LAST_RESULTS
    import ml_dtypes
    from concourse import bass_utils

    bfloat16 = ml_dtypes.bfloat16

    x = np.asarray(inputs["x"], dtype=np.float32)
    wpe = np.asarray(inputs["wpe"], dtype=np.float32)
    assert x.shape == (B, N, D), x.shape

    use_b1 = bool(np.any(np.asarray(inputs["mlp_b1"]) != 0))
    use_b2 = bool(np.any(np.asarray(inputs["mlp_b2"]) != 0))
    use_ln1 = not (np.all(np.asarray(inputs["ln1_g"]) == 1)
                   and np.all(np.asarray(inputs["ln1_b"]) == 0))
    use_ln2 = not (np.all(np.asarray(inputs["ln2_g"]) == 1)
                   and np.all(np.asarray(inputs["ln2_b"]) == 0))

    key = (use_b1, use_b2, use_ln1, use_ln2)
    if key not in _CACHE:
        _CACHE[key] = _build_program(*key)
    nc = _CACHE[key]

    h0 = x + wpe[None, :, :]  # positional embedding folded in on host

    tri = np.tril(np.ones((P, P), dtype=np.float32)).T  # tri[j,i] = j<=i
    ident = np.eye(P, dtype=np.float32)
    pos = np.arange(N, dtype=np.float32).reshape(NT, P).T  # [P, NT]
    invpos = (1.0 / (pos + 1.0)).astype(np.float32)

    shared = {
        "wq": np.ascontiguousarray(inputs["Wq"]).astype(bfloat16),
        "wk": np.ascontiguousarray(inputs["Wk"]).astype(bfloat16),
        "wv": np.ascontiguousarray(inputs["Wv"]).astype(bfloat16),
        "w1": np.ascontiguousarray(inputs["mlp_W1"]).astype(bfloat16),
        "w2": np.ascontiguousarray(inputs["mlp_W2"], dtype=np.float32),
        "tri": tri, "ident": ident.astype(bfloat16), "invpos": invpos,
    }
    if use_b1:
        shared["b1"] = np.asarray(inputs["mlp_b1"], dtype=np.float32)
    if use_b2:
        shared["b2"] = np.asarray(inputs["mlp_b2"], dtype=np.float32)
    if use_ln1:
        shared["ln1g"] = np.asarray(inputs["ln1_g"], dtype=np.float32)
        shared["ln1b"] = np.asarray(inputs["ln1_b"], dtype=np.float32)
    if use_ln2:
        shared["ln2g"] = np.asarray(inputs["ln2_g"], dtype=np.float32)
        shared["ln2b"] = np.asarray(inputs["ln2_b"], dtype=np.float32)

    in_maps = [dict(shared, h0=np.ascontiguousarray(h0[c]).astype(bfloat16))
               for c in range(B)]
    global _last_in_maps
    _last_in_maps = in_maps

    res = bass_utils.run_bass_kernel_spmd(
        nc, in_maps, core_ids=list(range(B)), trace=PROFILE)
    LAST_EXEC_NS = res.exec_time_ns
    LAST_RESULTS = res
    return np.stack(
        [np.asarray(res.results[c]["out"]).astype(np.float32)
         for c in range(B)], axis=0)


# revision 63
# speedup vs baseline: 1.0072x; 1.0072x over previous
"""Trainium2 Bass kernel for nn_DecoderTransformerBackbone_1589137900084.

Decoder transformer backbone: B=8, N=2048, D=256, L=4 layers of
relu-attention with a causal averaging mask + MLP, layernorms after each
residual. Data-parallel over batch: one batch element per NeuronCore (8 cores).

Per-core layout strategy (v2, restructured from the 335us baseline):
  - H (the residual stream) lives in SBUF as 16 tiles of [128 tokens, 256
    dims], dtype f32r (PE transposes run 1.5 cyc/row instead of f32's 2).
  - One shared PSUM pool of 8 rotating [128,512] single-bank slots feeds
    every phase (transposes, QKV, S, AV, MLP), maximizing the number of
    matmul outputs in flight so the PE never waits on PSUM drain copies.
  - Emission is interleaved so every PE instruction's inputs were produced
    several instructions earlier: T1 g0..g3 -> QK cp0 -> V p0,p1 -> QK cp1 ->
    S/AV ic0 -> V p2,p3 -> QK cp2 -> S/AV ic1 -> ... etc.
  - DMAs are ordered h0-first then layer-0 weights so the PE starts ~25us
    earlier than the load-everything-then-run baseline.
  - The in-block causal triangle multiply runs on the otherwise-idle GPSIMD
    engine; PSUM->SBUF drains alternate between Activation and DVE.
  - LN group stats: Sqrt activation (bias=eps, scale=1/D) + DVE reciprocal
    (bass blocks the Rsqrt table for accuracy reasons).
"""
import sys

sys.path.insert(0, "/opt/trn_rl_repo")

import numpy as np

B, N, D, L = 8, 2048, 256, 4
LN_EPS = 1e-5
P = 128
NT = N // P            # 16 token tiles
DT = D // P            # 2 dim tiles
IC = N // 512          # 4 free-dim chunks of 512

_CACHE = {}
_last_in_maps = None
REPEAT = 1
PROFILE = False
LAST_EXEC_NS = None
LAST_RESULTS = None


def _build_program(use_b1, use_b2, use_ln1_gb, use_ln2_gb):
    import concourse.bass as bass  # noqa: F401
    from concourse import bacc
    import concourse.mybir as mybir
    import concourse.tile as tile

    f32 = mybir.dt.float32
    f32r = mybir.dt.float32r
    bf16 = mybir.dt.bfloat16
    AF = mybir.ActivationFunctionType
    OP = mybir.AluOpType

    nc = bacc.Bacc("TRN2", target_bir_lowering=False)

    h0_d = nc.declare_dram_parameter("h0", [N, D], bf16, isOutput=False)
    wq_d = nc.declare_dram_parameter("wq", [L, D, D], bf16, isOutput=False)
    wk_d = nc.declare_dram_parameter("wk", [L, D, D], bf16, isOutput=False)
    wv_d = nc.declare_dram_parameter("wv", [L, D, D], bf16, isOutput=False)
    w1_d = nc.declare_dram_parameter("w1", [L, D, D], bf16, isOutput=False)
    w2_d = nc.declare_dram_parameter("w2", [L, D, D], f32r, isOutput=False)
    tri_d = nc.declare_dram_parameter("tri", [P, P], f32r, isOutput=False)
    invpos_d = nc.declare_dram_parameter("invpos", [P, NT], f32, isOutput=False)
    if use_b1:
        b1_d = nc.declare_dram_parameter("b1", [L, D], f32, isOutput=False)
    if use_b2:
        b2_d = nc.declare_dram_parameter("b2", [L, D], f32, isOutput=False)
    if use_ln1_gb:
        ln1g_d = nc.declare_dram_parameter("ln1g", [L, D], f32, isOutput=False)
        ln1b_d = nc.declare_dram_parameter("ln1b", [L, D], f32, isOutput=False)
    if use_ln2_gb:
        ln2g_d = nc.declare_dram_parameter("ln2g", [L, D], f32, isOutput=False)
        ln2b_d = nc.declare_dram_parameter("ln2b", [L, D], f32, isOutput=False)
    out_d = nc.declare_dram_parameter("out", [N, D], bf16, isOutput=True)

    with tile.TileContext(nc) as tc:
        with (
            tc.tile_pool(name="const", bufs=1) as constp,
            tc.tile_pool(name="work", bufs=1) as workp,
            tc.tile_pool(name="stp", bufs=20) as stp,
            tc.tile_pool(name="sqp", bufs=3) as sqp,
            tc.tile_pool(name="small", bufs=8) as smallp,
            tc.tile_pool(name="ps", bufs=8, space="PSUM") as psp,
        ):
            # ---------------- constants & activations (DMA order matters:
            # ident+h0 first so transposes start immediately, then layer-0
            # weights, then the rest) ----------------
            # build the transpose identity on-chip (no DMA on the critical
            # path): keep zeros where p != j, fill 1.0 on the diagonal
            ident = constp.tile([P, P], bf16, tag="ident")
            nc.vector.memset(ident[:], 0.0)
            nc.gpsimd.affine_select(
                out=ident[:], in_=ident[:], pattern=[[-1, P]],
                compare_op=OP.not_equal, fill=1.0,
                base=0, channel_multiplier=1)

            H = [workp.tile([P, D], bf16, tag=f"h_{nt}", name=f"h_{nt}")
                 for nt in range(NT)]

            def load_h0_group(g):
                # tiles 4g..4g+3 in one DMA each is impossible (separate
                # tiles), but 4 per-tile DMAs batch poorly on HWDGE; instead
                # two tiles per DMA via their adjacent pool slots is fragile,
                # so keep per-tile DMAs but emit them group-wise so weight
                # loads interleave between groups.
                for k in range(4):
                    nt = 4 * g + k
                    nc.sync.dma_start(H[nt][:], h0_d[nt * P:(nt + 1) * P, :])

            W = {}

            def load_w(wname, wd, l):
                dtype = f32r if wname == "w2" else bf16
                for dt_ in range(DT):
                    t = constp.tile([P, D], dtype, tag=f"{wname}_{l}_{dt_}")
                    nc.sync.dma_start(t[:], wd[l, dt_ * P:(dt_ + 1) * P, :])
                    W[wname, l, dt_] = t

            tri = constp.tile([P, P], f32r, tag="tri")
            invpos = constp.tile([P, NT], f32, tag="invpos")
            eps_t = constp.tile([P, 1], f32, tag="eps")

            load_h0_group(0)
            load_w("wq", wq_d, 0)
            load_w("wk", wk_d, 0)
            load_h0_group(1)
            load_w("wv", wv_d, 0)
            load_h0_group(2)
            load_w("w1", w1_d, 0)
            load_h0_group(3)
            load_w("w2", w2_d, 0)
            # tri/invpos ride the otherwise-empty Act-hosted DMA queue so
            # they land ~1.5us in (on the SP queue they sat behind all the
            # layer-0 weights and arrived after the first S drains need tri)
            nc.scalar.dma_start(tri[:], tri_d[:])
            nc.scalar.dma_start(invpos[:], invpos_d[:])
            for l in range(1, L):
                for wname, wd in (("wq", wq_d), ("wk", wk_d), ("wv", wv_d),
                                  ("w1", w1_d), ("w2", w2_d)):
                    load_w(wname, wd, l)
            nc.vector.memset(eps_t[:], LN_EPS)

            def load_vec(dram, tag):
                out = []
                for l in range(L):
                    t = constp.tile([P, DT], f32, tag=f"{tag}_{l}")
                    nc.sync.dma_start(
                        t[:], dram[l].rearrange("(dt p) -> p dt", p=P))
                    out.append(t)
                return out

            def load_bcast(dram, tag):
                out = []
                for l in range(L):
                    t = constp.tile([P, D], f32, tag=f"{tag}b_{l}")
                    nc.sync.dma_start(
                        t[:], dram[l].unsqueeze(0).to_broadcast([P, D]))
                    out.append(t)
                return out

            b1_t = load_vec(b1_d, "b1") if use_b1 else None
            b2_t = load_bcast(b2_d, "b2") if use_b2 else None
            ln1g_t = load_bcast(ln1g_d, "ln1g") if use_ln1_gb else None
            ln1b_t = load_bcast(ln1b_d, "ln1b") if use_ln1_gb else None
            ln2g_t = load_bcast(ln2g_d, "ln2g") if use_ln2_gb else None
            ln2b_t = load_bcast(ln2b_d, "ln2b") if use_ln2_gb else None

            qT = [workp.tile([P, N], bf16, tag=f"qt_{d}", name=f"qt_{d}")
                  for d in range(DT)]
            kT = [workp.tile([P, N], bf16, tag=f"kt_{d}", name=f"kt_{d}")
                  for d in range(DT)]
            h1T = [workp.tile([P, N], f32r, tag=f"h1t_{d}", name=f"h1t_{d}")
                   for d in range(DT)]
            HT = [[workp.tile([P, N], bf16, tag=f"ht_{d}", name=f"ht_{d}")
                   for d in range(DT)]]
            Vp = [[workp.tile([P, 2 * D], f32r, tag=f"vp_{i}", name=f"vp_{i}")
                   for i in range(NT // 2)]]

            def Vsl(nt):
                return Vp[0][nt // 2][:, (nt % 2) * D:(nt % 2 + 1) * D]

            def ps_slot(name, dtype=f32):
                return psp.tile([P, 512], dtype, tag="ps", name=name)

            _alt = [0]

            def psum_copy(dst, src, relu=False):
                # alternate Act / DVE for PSUM->SBUF drains
                _alt[0] ^= 1
                if _alt[0]:
                    nc.scalar.activation(dst, src, AF.Relu if relu else AF.Copy)
                elif relu:
                    nc.vector.tensor_scalar_max(dst, src, 0.0)
                else:
                    nc.vector.tensor_copy(dst, src)

            # ---------------- per-phase emitters ----------------
            def emit_T_pair(pair, dst):
                # transpose H tiles 2*pair..2*pair+1 into dst cols pair*256..
                for dt_ in range(DT):
                    ps = ps_slot("ps_t", bf16)
                    for k in range(2):
                        nt = 2 * pair + k
                        nc.tensor.transpose(
                            ps[:, k * P:(k + 1) * P],
                            H[nt][:, dt_ * P:(dt_ + 1) * P],
                            ident[:],
                        )
                    psum_copy(dst[0][dt_][:, pair * 256:(pair + 1) * 256],
                              ps[:, :256])

            def emit_h1_half(hh, l):
                # 256-wide h1 chunk consuming T2 pair hh (cols hh*256..)
                ps = ps_slot("ps_h1h")
                for et in range(DT):
                    for dt_ in range(DT):
                        nc.tensor.matmul(
                            ps[:, et * 256:(et + 1) * 256],
                            W["w1", l, dt_][:, et * P:(et + 1) * P],
                            HT[0][dt_][:, hh * 256:(hh + 1) * 256],
                            start=(dt_ == 0), stop=(dt_ == DT - 1),
                            skip_group_check=True,
                        )
                for et in range(DT):
                    dst = h1T[et][:, hh * 256:(hh + 1) * 256]
                    if use_b1:
                        nc.scalar.activation(dst, ps[:, et * 256:(et + 1) * 256],
                                             AF.Relu, bias=b1_t[l][:, et:et + 1])
                    elif et == 0:
                        nc.scalar.activation(dst, ps[:, et * 256:(et + 1) * 256],
                                             AF.Relu)
                    else:
                        nc.vector.tensor_scalar_max(
                            dst, ps[:, et * 256:(et + 1) * 256], 0.0)

            def emit_qk_half(half, wname, dst, l):
                # 256-wide first-chunk qk consuming one transpose pair;
                # both et halves share one psum slot
                ps = ps_slot("ps_qk")
                for et in range(DT):
                    for dt_ in range(DT):
                        nc.tensor.matmul(
                            ps[:, et * 256:(et + 1) * 256],
                            W[wname, l, dt_][:, et * P:(et + 1) * P],
                            HT[0][dt_][:, half * 256:(half + 1) * 256],
                            start=(dt_ == 0), stop=(dt_ == DT - 1),
                            skip_group_check=True,
                        )
                for et in range(DT):
                    psum_copy(dst[et][:, half * 256:(half + 1) * 256],
                              ps[:, et * 256:(et + 1) * 256])

            def emit_T(g, dst):
                # transpose H tiles 4g..4g+3 into dst[:, g*512:(g+1)*512]
                for dt_ in range(DT):
                    ps = ps_slot("ps_t", bf16)
                    for k in range(4):
                        nt = 4 * g + k
                        nc.tensor.transpose(
                            ps[:, k * P:(k + 1) * P],
                            H[nt][:, dt_ * P:(dt_ + 1) * P],
                            ident[:],
                        )
                    psum_copy(dst[0][dt_][:, g * 512:(g + 1) * 512], ps[:])

            def emit_qk(cp, wname, dst, l):
                # produce dst[et][:, cp*512:(cp+1)*512] for et in 0,1
                for et in range(DT):
                    ps = ps_slot("ps_qk")
                    for dt_ in range(DT):
                        nc.tensor.matmul(
                            ps[:],
                            W[wname, l, dt_][:, et * P:(et + 1) * P],
                            HT[0][dt_][:, cp * 512:(cp + 1) * 512],
                            start=(dt_ == 0), stop=(dt_ == DT - 1),
                            skip_group_check=True,
                        )
                    psum_copy(dst[et][:, cp * 512:(cp + 1) * 512], ps[:])

            def emit_v(pair, l):
                ps = ps_slot("ps_v")
                for k in range(2):
                    nt = 2 * pair + k
                    for dt_ in range(DT):
                        nc.tensor.matmul(
                            ps[:, k * D:(k + 1) * D],
                            HT[0][dt_][:, nt * P:(nt + 1) * P],
                            W["wv", l, dt_][:],
                            start=(dt_ == 0), stop=(dt_ == DT - 1),
                            skip_group_check=True,
                        )
                psum_copy(Vp[0][pair][:], ps[:])

            def ln_group_stats(sum_g, ssq_g, n, tag):
                # rstd = rsqrt((ssq - sum^2/D)/D + eps); nmu = -sum/D*rstd
                sq = smallp.tile([P, 4], f32, tag=f"lnsq{tag}", name="lnsq")
                rstd = smallp.tile([P, 4], f32, tag=f"lnrstd{tag}", name="lnrstd")
                nmu = smallp.tile([P, 4], f32, tag=f"lnnmu{tag}", name="lnnmu")
                nc.vector.tensor_tensor(out=sq[:, :n], in0=sum_g[:, :n],
                                        in1=sum_g[:, :n], op=OP.mult)
                nc.vector.scalar_tensor_tensor(
                    out=sq[:, :n], in0=sq[:, :n], scalar=-1.0 / D,
                    in1=ssq_g[:, :n], op0=OP.mult, op1=OP.add)
                nc.scalar.activation(rstd[:, :n], sq[:, :n], AF.Sqrt,
                                     bias=eps_t[:], scale=1.0 / D)
                nc.vector.reciprocal(rstd[:, :n], rstd[:, :n])
                nc.vector.scalar_tensor_tensor(
                    out=nmu[:, :n], in0=sum_g[:, :n], scalar=-1.0 / D,
                    in1=rstd[:, :n], op0=OP.mult, op1=OP.mult)
                return rstd, nmu

            def ln_sumsq(h, dst, k):
                if k % 2 == 0:
                    sq = sqp.tile([P, D], bf16, tag="sqs", name="sqs")
                    nc.scalar.activation(sq[:], h[:], AF.Square, accum_out=dst)
                else:
                    sq = sqp.tile([P, D], bf16, tag="sqv", name="sqv")
                    nc.vector.scalar_tensor_tensor(
                        out=sq[:], in0=h[:], scalar=1.0, in1=h[:],
                        op0=OP.mult, op1=OP.mult, accum_out=dst)

            def ln_apply(h, rstd, nmu, k, g_t, b_t):
                if k % 2 == 0:
                    nc.scalar.activation(h[:], h[:], AF.Identity,
                                         scale=rstd[:, k:k + 1],
                                         bias=nmu[:, k:k + 1])
                else:
                    nc.vector.tensor_scalar(
                        out=h[:], in0=h[:], scalar1=rstd[:, k:k + 1],
                        scalar2=nmu[:, k:k + 1], op0=OP.mult, op1=OP.add)
                if g_t is not None:
                    nc.vector.tensor_tensor(out=h[:], in0=h[:], in1=g_t[:],
                                            op=OP.mult)
                    nc.vector.tensor_tensor(out=h[:], in0=h[:], in1=b_t[:],
                                            op=OP.add)

            def emit_s_block(ic, jt, STl):
                c0 = P * max(0, jt - 4 * ic)
                # qT/kT are bf16 (1 cyc/row at ANY width), so diagonal blocks
                # can run at their exact width - no fp32r >=256 clamp
                c0p = c0
                ps = ps_slot("ps_s")
                for et in range(DT):
                    nc.tensor.matmul(
                        ps[:, c0p:],
                        kT[et][:, jt * P:(jt + 1) * P],
                        qT[et][:, ic * 512 + c0p:(ic + 1) * 512],
                        start=(et == 0), stop=(et == DT - 1),
                    )
                st = stp.tile([P, 512], f32r, tag="st", name="st")
                psum_copy(st[:, c0:], ps[:, c0:], relu=True)
                if jt >= 4 * ic:
                    nc.gpsimd.tensor_tensor(
                        out=st[:, c0:c0 + P], in0=st[:, c0:c0 + P],
                        in1=tri[:], op=OP.mult)
                STl.append(st)

            def emit_av(ib, STl, sum_g, ib_l):
                ps = ps_slot("ps_av")
                for jt in range(ib + 1):
                    nc.tensor.matmul(
                        ps[:, :D],
                        STl[jt][:, ib_l * P:(ib_l + 1) * P],
                        Vsl(jt),
                        start=(jt == 0), stop=(jt == ib),
                    )
                nc.vector.scalar_tensor_tensor(
                    out=H[ib][:], in0=ps[:, :D],
                    scalar=invpos[:, ib:ib + 1], in1=H[ib][:],
                    op0=OP.mult, op1=OP.add,
                    accum_out=sum_g[:, ib_l:ib_l + 1])

            def emit_h1(cp, l):
                for et in range(DT):
                    ps = ps_slot("ps_h1")
                    for dt_ in range(DT):
                        nc.tensor.matmul(
                            ps[:],
                            W["w1", l, dt_][:, et * P:(et + 1) * P],
                            HT[0][dt_][:, cp * 512:(cp + 1) * 512],
                            start=(dt_ == 0), stop=(dt_ == DT - 1),
                            skip_group_check=True,
                        )
                    dst = h1T[et][:, cp * 512:(cp + 1) * 512]
                    _alt[0] ^= 1
                    if use_b1:
                        nc.scalar.activation(dst, ps[:], AF.Relu,
                                             bias=b1_t[l][:, et:et + 1])
                    elif _alt[0]:
                        nc.scalar.activation(dst, ps[:], AF.Relu)
                    else:
                        nc.vector.tensor_scalar_max(dst, ps[:], 0.0)

            def emit_m2(nt, l, sum_g, k):
                ps = ps_slot("ps_m2")
                for et in range(DT):
                    nc.tensor.matmul(
                        ps[:, :D],
                        h1T[et][:, nt * P:(nt + 1) * P],
                        W["w2", l, et][:],
                        start=(et == 0), stop=(et == DT - 1),
                    )
                if use_b2:
                    nc.vector.scalar_tensor_tensor(
                        out=H[nt][:], in0=ps[:, :D], scalar=1.0,
                        in1=H[nt][:], op0=OP.mult, op1=OP.add)
                    nc.vector.tensor_tensor(out=H[nt][:], in0=H[nt][:],
                                            in1=b2_t[l][:], op=OP.add)
                    nc.vector.tensor_reduce(
                        out=sum_g[:, k:k + 1], in_=H[nt][:],
                        axis=mybir.AxisListType.X, op=OP.add)
                else:
                    nc.vector.scalar_tensor_tensor(
                        out=H[nt][:], in0=ps[:, :D], scalar=1.0,
                        in1=H[nt][:], op0=OP.mult, op1=OP.add,
                        accum_out=sum_g[:, k:k + 1])

            # ---------------- main layer loop ----------------
            for li in range(L * REPEAT):
                l = li % L

                sum1 = smallp.tile([P, NT], f32, tag="sum1", name="sum1")
                ssq1 = smallp.tile([P, NT], f32, tag="ssq1", name="ssq1")
                sum2 = smallp.tile([P, NT], f32, tag="sum2", name="sum2")
                ssq2 = smallp.tile([P, NT], f32, tag="ssq2", name="ssq2")

                def emit_s_chunk(ic):
                    STl = []
                    for jt in range(4 * ic + 4):
                        emit_s_block(ic, jt, STl)
                    return STl

                def emit_av_ln(ic, STl):
                    for ib_l in range(4):
                        ib = 4 * ic + ib_l
                        emit_av(ib, STl, sum1[:, 4 * ic:4 * ic + 4], ib_l)
                        ln_sumsq(H[ib], ssq1[:, ib:ib + 1], ib_l)
                    rstd, nmu = ln_group_stats(
                        sum1[:, 4 * ic:4 * ic + 4], ssq1[:, 4 * ic:4 * ic + 4],
                        4, "1")
                    for ib_l in range(4):
                        ib = 4 * ic + ib_l
                        ln_apply(H[ib], rstd, nmu, ib_l,
                                 ln1g_t[l] if use_ln1_gb else None,
                                 ln1b_t[l] if use_ln1_gb else None)

                def mlp_chunk(g):
                    for k in range(4):
                        nt = 4 * g + k
                        emit_m2(nt, l, sum2[:, 4 * g:4 * g + 4], k)
                        ln_sumsq(H[nt], ssq2[:, nt:nt + 1], k)
                    rstd, nmu = ln_group_stats(
                        sum2[:, 4 * g:4 * g + 4], ssq2[:, 4 * g:4 * g + 4],
                        4, "2")
                    for k in range(4):
                        nt = 4 * g + k
                        ln_apply(H[nt], rstd, nmu, k,
                                 ln2g_t[l] if use_ln2_gb else None,
                                 ln2b_t[l] if use_ln2_gb else None)
                        if li == L * REPEAT - 1:
                            nc.sync.dma_start(
                                out_d[nt * P:(nt + 1) * P, :], H[nt][:])

                # --- phase A: transposes + QKV interleaved with S/AV ---
                # pair-granular start: qk(cp0) halves consume transpose pairs
                # as soon as their two LN applies land
                emit_T_pair(0, HT)
                emit_qk_half(0, "wq", qT, l)
                emit_T_pair(1, HT)
                emit_qk_half(0, "wk", kT, l)
                emit_qk_half(1, "wq", qT, l)
                emit_qk_half(1, "wk", kT, l)
                emit_v(0, l)
                emit_T(1, HT)
                emit_v(1, l)
                st0 = emit_s_chunk(0)
                emit_T(2, HT)
                emit_qk(1, "wq", qT, l)
                emit_qk(1, "wk", kT, l)
                emit_T(3, HT)
                emit_av_ln(0, st0)
                emit_v(2, l)
                emit_v(3, l)
                st1 = emit_s_chunk(1)
                emit_qk(2, "wq", qT, l)
                emit_qk(2, "wk", kT, l)
                emit_av_ln(1, st1)
                emit_v(4, l)
                emit_v(5, l)
                st2 = emit_s_chunk(2)
                emit_qk(3, "wq", qT, l)
                emit_qk(3, "wk", kT, l)
                emit_v(6, l)
                emit_v(7, l)
                emit_av_ln(2, st2)
                # --- phase B: MLP (T2/h1 interleave into the attn tail) ---
                st3 = emit_s_chunk(3)
                emit_T(0, HT)
                emit_av_ln(3, st3)
                emit_h1(0, l)
                emit_T(1, HT)
                emit_h1(1, l)
                emit_T(2, HT)
                mlp_chunk(0)
                emit_h1(2, l)
                emit_T(3, HT)
                mlp_chunk(1)
                emit_h1(3, l)
                mlp_chunk(2)
                mlp_chunk(3)

    nc.finalize()
    return nc


def kernel(**inputs):
    global LAST_EXEC_NS, LAST_RESULTS
    import ml_dtypes
    from concourse import bass_utils

    bfloat16 = ml_dtypes.bfloat16

    x = np.asarray(inputs["x"], dtype=np.float32)
    wpe = np.asarray(inputs["wpe"], dtype=np.float32)
    assert x.shape == (B, N, D), x.shape

    use_b1 = bool(np.any(np.asarray(inputs["mlp_b1"]) != 0))
    use_b2 = bool(np.any(np.asarray(inputs["mlp_b2"]) != 0))
    use_ln1 = not (np.all(np.asarray(inputs["ln1_g"]) == 1)
                   and np.all(np.asarray(inputs["ln1_b"]) == 0))
    use_ln2 = not (np.all(np.asarray(inputs["ln2_g"]) == 1)
                   and np.all(np.asarray(inputs["ln2_b"]) == 0))

    key = (use_b1, use_b2, use_ln1, use_ln2)
    if key not in _CACHE:
        _CACHE[key] = _build_program(*key)
    nc = _CACHE[key]

    h0 = x + wpe[None, :, :]  # positional embedding folded in on host

    tri = np.tril(np.ones((P, P), dtype=np.float32)).T  # tri[j,i] = j<=i
    ident = np.eye(P, dtype=np.float32)
    pos = np.arange(N, dtype=np.float32).reshape(NT, P).T  # [P, NT]
    invpos = (1.0 / (pos + 1.0)).astype(np.float32)

    shared = {
        "wq": np.ascontiguousarray(inputs["Wq"]).astype(bfloat16),
        "wk": np.ascontiguousarray(inputs["Wk"]).astype(bfloat16),
        "wv": np.ascontiguousarray(inputs["Wv"]).astype(bfloat16),
        "w1": np.ascontiguousarray(inputs["mlp_W1"]).astype(bfloat16),
        "w2": np.ascontiguousarray(inputs["mlp_W2"], dtype=np.float32),
        "tri": tri, "ident": ident.astype(bfloat16), "invpos": invpos,
    }
    if use_b1:
        shared["b1"] = np.asarray(inputs["mlp_b1"], dtype=np.float32)
    if use_b2:
        shared["b2"] = np.asarray(inputs["mlp_b2"], dtype=np.float32)
    if use_ln1:
        shared["ln1g"] = np.asarray(inputs["ln1_g"], dtype=np.float32)
        shared["ln1b"] = np.asarray(inputs["ln1_b"], dtype=np.float32)
    if use_ln2:
        shared["ln2g"] = np.asarray(inputs["ln2_g"], dtype=np.float32)
        shared["ln2b"] = np.asarray(inputs["ln2_b"], dtype=np.float32)

    in_maps = [dict(shared, h0=np.ascontiguousarray(h0[c]).astype(bfloat16))
               for c in range(B)]
    global _last_in_maps
    _last_in_maps = in_maps

    res = bass_utils.run_bass_kernel_spmd(
        nc, in_maps, core_ids=list(range(B)), trace=PROFILE)
    LAST_EXEC_NS = res.exec_time_ns
    LAST_RESULTS = res
    return np.stack(
        [np.asarray(res.results[c]["out"]).astype(np.float32)
         for c in range(B)], axis=0)


# revision 64
# speedup vs baseline: 1.0092x; 1.0020x over previous
"""Trainium2 Bass kernel for nn_DecoderTransformerBackbone_1589137900084.

Decoder transformer backbone: B=8, N=2048, D=256, L=4 layers of
relu-attention with a causal averaging mask + MLP, layernorms after each
residual. Data-parallel over batch: one batch element per NeuronCore (8 cores).

Per-core layout strategy (v2, restructured from the 335us baseline):
  - H (the residual stream) lives in SBUF as 16 tiles of [128 tokens, 256
    dims], dtype f32r (PE transposes run 1.5 cyc/row instead of f32's 2).
  - One shared PSUM pool of 8 rotating [128,512] single-bank slots feeds
    every phase (transposes, QKV, S, AV, MLP), maximizing the number of
    matmul outputs in flight so the PE never waits on PSUM drain copies.
  - Emission is interleaved so every PE instruction's inputs were produced
    several instructions earlier: T1 g0..g3 -> QK cp0 -> V p0,p1 -> QK cp1 ->
    S/AV ic0 -> V p2,p3 -> QK cp2 -> S/AV ic1 -> ... etc.
  - DMAs are ordered h0-first then layer-0 weights so the PE starts ~25us
    earlier than the load-everything-then-run baseline.
  - The in-block causal triangle multiply runs on the otherwise-idle GPSIMD
    engine; PSUM->SBUF drains alternate between Activation and DVE.
  - LN group stats: Sqrt activation (bias=eps, scale=1/D) + DVE reciprocal
    (bass blocks the Rsqrt table for accuracy reasons).
"""
import sys

sys.path.insert(0, "/opt/trn_rl_repo")

import numpy as np

B, N, D, L = 8, 2048, 256, 4
LN_EPS = 1e-5
P = 128
NT = N // P            # 16 token tiles
DT = D // P            # 2 dim tiles
IC = N // 512          # 4 free-dim chunks of 512

_CACHE = {}
_last_in_maps = None
REPEAT = 1
PROFILE = False
LAST_EXEC_NS = None
LAST_RESULTS = None


def _build_program(use_b1, use_b2, use_ln1_gb, use_ln2_gb):
    import concourse.bass as bass  # noqa: F401
    from concourse import bacc
    import concourse.mybir as mybir
    import concourse.tile as tile

    f32 = mybir.dt.float32
    f32r = mybir.dt.float32r
    bf16 = mybir.dt.bfloat16
    AF = mybir.ActivationFunctionType
    OP = mybir.AluOpType

    nc = bacc.Bacc("TRN2", target_bir_lowering=False)

    h0_d = nc.declare_dram_parameter("h0", [N, D], bf16, isOutput=False)
    wq_d = nc.declare_dram_parameter("wq", [L, D, D], bf16, isOutput=False)
    wk_d = nc.declare_dram_parameter("wk", [L, D, D], bf16, isOutput=False)
    wv_d = nc.declare_dram_parameter("wv", [L, D, D], bf16, isOutput=False)
    w1_d = nc.declare_dram_parameter("w1", [L, D, D], bf16, isOutput=False)
    w2_d = nc.declare_dram_parameter("w2", [L, D, D], f32r, isOutput=False)
    tri_d = nc.declare_dram_parameter("tri", [P, P], f32r, isOutput=False)
    invpos_d = nc.declare_dram_parameter("invpos", [P, NT], f32, isOutput=False)
    if use_b1:
        b1_d = nc.declare_dram_parameter("b1", [L, D], f32, isOutput=False)
    if use_b2:
        b2_d = nc.declare_dram_parameter("b2", [L, D], f32, isOutput=False)
    if use_ln1_gb:
        ln1g_d = nc.declare_dram_parameter("ln1g", [L, D], f32, isOutput=False)
        ln1b_d = nc.declare_dram_parameter("ln1b", [L, D], f32, isOutput=False)
    if use_ln2_gb:
        ln2g_d = nc.declare_dram_parameter("ln2g", [L, D], f32, isOutput=False)
        ln2b_d = nc.declare_dram_parameter("ln2b", [L, D], f32, isOutput=False)
    out_d = nc.declare_dram_parameter("out", [N, D], bf16, isOutput=True)

    with tile.TileContext(nc) as tc:
        with (
            tc.tile_pool(name="const", bufs=1) as constp,
            tc.tile_pool(name="work", bufs=1) as workp,
            tc.tile_pool(name="stp", bufs=20) as stp,
            tc.tile_pool(name="sqp", bufs=3) as sqp,
            tc.tile_pool(name="small", bufs=8) as smallp,
            tc.tile_pool(name="ps", bufs=8, space="PSUM") as psp,
        ):
            # ---------------- constants & activations (DMA order matters:
            # ident+h0 first so transposes start immediately, then layer-0
            # weights, then the rest) ----------------
            # build the transpose identity on-chip (no DMA on the critical
            # path): keep zeros where p != j, fill 1.0 on the diagonal
            ident = constp.tile([P, P], bf16, tag="ident")
            nc.vector.memset(ident[:], 0.0)
            nc.gpsimd.affine_select(
                out=ident[:], in_=ident[:], pattern=[[-1, P]],
                compare_op=OP.not_equal, fill=1.0,
                base=0, channel_multiplier=1)

            H = [workp.tile([P, D], bf16, tag=f"h_{nt}", name=f"h_{nt}")
                 for nt in range(NT)]

            def load_h0_group(g):
                # tiles 4g..4g+3 in one DMA each is impossible (separate
                # tiles), but 4 per-tile DMAs batch poorly on HWDGE; instead
                # two tiles per DMA via their adjacent pool slots is fragile,
                # so keep per-tile DMAs but emit them group-wise so weight
                # loads interleave between groups.
                for k in range(4):
                    nt = 4 * g + k
                    nc.sync.dma_start(H[nt][:], h0_d[nt * P:(nt + 1) * P, :])

            W = {}

            def load_w(wname, wd, l, eng=None):
                dtype = f32r if wname == "w2" else bf16
                eng = eng or nc.sync
                for dt_ in range(DT):
                    t = constp.tile([P, D], dtype, tag=f"{wname}_{l}_{dt_}")
                    eng.dma_start(t[:], wd[l, dt_ * P:(dt_ + 1) * P, :])
                    W[wname, l, dt_] = t

            tri = constp.tile([P, P], f32r, tag="tri")
            invpos = constp.tile([P, NT], f32, tag="invpos")
            eps_t = constp.tile([P, 1], f32, tag="eps")

            load_h0_group(0)
            load_w("wq", wq_d, 0)
            load_w("wk", wk_d, 0)
            load_h0_group(1)
            load_w("wv", wv_d, 0, eng=nc.scalar)
            load_h0_group(2)
            load_w("w1", w1_d, 0)
            load_h0_group(3)
            load_w("w2", w2_d, 0)
            # tri/invpos ride the otherwise-empty Act-hosted DMA queue so
            # they land ~1.5us in (on the SP queue they sat behind all the
            # layer-0 weights and arrived after the first S drains need tri)
            nc.scalar.dma_start(tri[:], tri_d[:])
            nc.scalar.dma_start(invpos[:], invpos_d[:])
            for l in range(1, L):
                for wname, wd in (("wq", wq_d), ("wk", wk_d), ("wv", wv_d),
                                  ("w1", w1_d), ("w2", w2_d)):
                    load_w(wname, wd, l)
            nc.vector.memset(eps_t[:], LN_EPS)

            def load_vec(dram, tag):
                out = []
                for l in range(L):
                    t = constp.tile([P, DT], f32, tag=f"{tag}_{l}")
                    nc.sync.dma_start(
                        t[:], dram[l].rearrange("(dt p) -> p dt", p=P))
                    out.append(t)
                return out

            def load_bcast(dram, tag):
                out = []
                for l in range(L):
                    t = constp.tile([P, D], f32, tag=f"{tag}b_{l}")
                    nc.sync.dma_start(
                        t[:], dram[l].unsqueeze(0).to_broadcast([P, D]))
                    out.append(t)
                return out

            b1_t = load_vec(b1_d, "b1") if use_b1 else None
            b2_t = load_bcast(b2_d, "b2") if use_b2 else None
            ln1g_t = load_bcast(ln1g_d, "ln1g") if use_ln1_gb else None
            ln1b_t = load_bcast(ln1b_d, "ln1b") if use_ln1_gb else None
            ln2g_t = load_bcast(ln2g_d, "ln2g") if use_ln2_gb else None
            ln2b_t = load_bcast(ln2b_d, "ln2b") if use_ln2_gb else None

            qT = [workp.tile([P, N], bf16, tag=f"qt_{d}", name=f"qt_{d}")
                  for d in range(DT)]
            kT = [workp.tile([P, N], bf16, tag=f"kt_{d}", name=f"kt_{d}")
                  for d in range(DT)]
            h1T = [workp.tile([P, N], f32r, tag=f"h1t_{d}", name=f"h1t_{d}")
                   for d in range(DT)]
            HT = [[workp.tile([P, N], bf16, tag=f"ht_{d}", name=f"ht_{d}")
                   for d in range(DT)]]
            Vp = [[workp.tile([P, 2 * D], f32r, tag=f"vp_{i}", name=f"vp_{i}")
                   for i in range(NT // 2)]]

            def Vsl(nt):
                return Vp[0][nt // 2][:, (nt % 2) * D:(nt % 2 + 1) * D]

            def ps_slot(name, dtype=f32):
                return psp.tile([P, 512], dtype, tag="ps", name=name)

            _alt = [0]

            def psum_copy(dst, src, relu=False):
                # alternate Act / DVE for PSUM->SBUF drains
                _alt[0] ^= 1
                if _alt[0]:
                    nc.scalar.activation(dst, src, AF.Relu if relu else AF.Copy)
                elif relu:
                    nc.vector.tensor_scalar_max(dst, src, 0.0)
                else:
                    nc.vector.tensor_copy(dst, src)

            # ---------------- per-phase emitters ----------------
            def emit_T_pair(pair, dst):
                # transpose H tiles 2*pair..2*pair+1 into dst cols pair*256..
                for dt_ in range(DT):
                    ps = ps_slot("ps_t", bf16)
                    for k in range(2):
                        nt = 2 * pair + k
                        nc.tensor.transpose(
                            ps[:, k * P:(k + 1) * P],
                            H[nt][:, dt_ * P:(dt_ + 1) * P],
                            ident[:],
                        )
                    psum_copy(dst[0][dt_][:, pair * 256:(pair + 1) * 256],
                              ps[:, :256])

            def emit_h1_half(hh, l):
                # 256-wide h1 chunk consuming T2 pair hh (cols hh*256..)
                ps = ps_slot("ps_h1h")
                for et in range(DT):
                    for dt_ in range(DT):
                        nc.tensor.matmul(
                            ps[:, et * 256:(et + 1) * 256],
                            W["w1", l, dt_][:, et * P:(et + 1) * P],
                            HT[0][dt_][:, hh * 256:(hh + 1) * 256],
                            start=(dt_ == 0), stop=(dt_ == DT - 1),
                            skip_group_check=True,
                        )
                for et in range(DT):
                    dst = h1T[et][:, hh * 256:(hh + 1) * 256]
                    if use_b1:
                        nc.scalar.activation(dst, ps[:, et * 256:(et + 1) * 256],
                                             AF.Relu, bias=b1_t[l][:, et:et + 1])
                    elif et == 0:
                        nc.scalar.activation(dst, ps[:, et * 256:(et + 1) * 256],
                                             AF.Relu)
                    else:
                        nc.vector.tensor_scalar_max(
                            dst, ps[:, et * 256:(et + 1) * 256], 0.0)

            def emit_qk_half(half, wname, dst, l):
                # 256-wide first-chunk qk consuming one transpose pair;
                # both et halves share one psum slot
                ps = ps_slot("ps_qk")
                for et in range(DT):
                    for dt_ in range(DT):
                        nc.tensor.matmul(
                            ps[:, et * 256:(et + 1) * 256],
                            W[wname, l, dt_][:, et * P:(et + 1) * P],
                            HT[0][dt_][:, half * 256:(half + 1) * 256],
                            start=(dt_ == 0), stop=(dt_ == DT - 1),
                            skip_group_check=True,
                        )
                for et in range(DT):
                    psum_copy(dst[et][:, half * 256:(half + 1) * 256],
                              ps[:, et * 256:(et + 1) * 256])

            def emit_T(g, dst):
                # transpose H tiles 4g..4g+3 into dst[:, g*512:(g+1)*512]
                for dt_ in range(DT):
                    ps = ps_slot("ps_t", bf16)
                    for k in range(4):
                        nt = 4 * g + k
                        nc.tensor.transpose(
                            ps[:, k * P:(k + 1) * P],
                            H[nt][:, dt_ * P:(dt_ + 1) * P],
                            ident[:],
                        )
                    psum_copy(dst[0][dt_][:, g * 512:(g + 1) * 512], ps[:])

            def emit_qk(cp, wname, dst, l):
                # produce dst[et][:, cp*512:(cp+1)*512] for et in 0,1
                for et in range(DT):
                    ps = ps_slot("ps_qk")
                    for dt_ in range(DT):
                        nc.tensor.matmul(
                            ps[:],
                            W[wname, l, dt_][:, et * P:(et + 1) * P],
                            HT[0][dt_][:, cp * 512:(cp + 1) * 512],
                            start=(dt_ == 0), stop=(dt_ == DT - 1),
                            skip_group_check=True,
                        )
                    psum_copy(dst[et][:, cp * 512:(cp + 1) * 512], ps[:])

            def emit_v(pair, l):
                ps = ps_slot("ps_v")
                for k in range(2):
                    nt = 2 * pair + k
                    for dt_ in range(DT):
                        nc.tensor.matmul(
                            ps[:, k * D:(k + 1) * D],
                            HT[0][dt_][:, nt * P:(nt + 1) * P],
                            W["wv", l, dt_][:],
                            start=(dt_ == 0), stop=(dt_ == DT - 1),
                            skip_group_check=True,
                        )
                psum_copy(Vp[0][pair][:], ps[:])

            def ln_group_stats(sum_g, ssq_g, n, tag):
                # rstd = rsqrt((ssq - sum^2/D)/D + eps); nmu = -sum/D*rstd
                sq = smallp.tile([P, 4], f32, tag=f"lnsq{tag}", name="lnsq")
                rstd = smallp.tile([P, 4], f32, tag=f"lnrstd{tag}", name="lnrstd")
                nmu = smallp.tile([P, 4], f32, tag=f"lnnmu{tag}", name="lnnmu")
                nc.vector.tensor_tensor(out=sq[:, :n], in0=sum_g[:, :n],
                                        in1=sum_g[:, :n], op=OP.mult)
                nc.vector.scalar_tensor_tensor(
                    out=sq[:, :n], in0=sq[:, :n], scalar=-1.0 / D,
                    in1=ssq_g[:, :n], op0=OP.mult, op1=OP.add)
                nc.scalar.activation(rstd[:, :n], sq[:, :n], AF.Sqrt,
                                     bias=eps_t[:], scale=1.0 / D)
                nc.vector.reciprocal(rstd[:, :n], rstd[:, :n])
                nc.vector.scalar_tensor_tensor(
                    out=nmu[:, :n], in0=sum_g[:, :n], scalar=-1.0 / D,
                    in1=rstd[:, :n], op0=OP.mult, op1=OP.mult)
                return rstd, nmu

            def ln_sumsq(h, dst, k):
                if k % 2 == 0:
                    sq = sqp.tile([P, D], bf16, tag="sqs", name="sqs")
                    nc.scalar.activation(sq[:], h[:], AF.Square, accum_out=dst)
                else:
                    sq = sqp.tile([P, D], bf16, tag="sqv", name="sqv")
                    nc.vector.scalar_tensor_tensor(
                        out=sq[:], in0=h[:], scalar=1.0, in1=h[:],
                        op0=OP.mult, op1=OP.mult, accum_out=dst)

            def ln_apply(h, rstd, nmu, k, g_t, b_t):
                if k % 2 == 0:
                    nc.scalar.activation(h[:], h[:], AF.Identity,
                                         scale=rstd[:, k:k + 1],
                                         bias=nmu[:, k:k + 1])
                else:
                    nc.vector.tensor_scalar(
                        out=h[:], in0=h[:], scalar1=rstd[:, k:k + 1],
                        scalar2=nmu[:, k:k + 1], op0=OP.mult, op1=OP.add)
                if g_t is not None:
                    nc.vector.tensor_tensor(out=h[:], in0=h[:], in1=g_t[:],
                                            op=OP.mult)
                    nc.vector.tensor_tensor(out=h[:], in0=h[:], in1=b_t[:],
                                            op=OP.add)

            def emit_s_block(ic, jt, STl):
                c0 = P * max(0, jt - 4 * ic)
                # qT/kT are bf16 (1 cyc/row at ANY width), so diagonal blocks
                # can run at their exact width - no fp32r >=256 clamp
                c0p = c0
                ps = ps_slot("ps_s")
                for et in range(DT):
                    nc.tensor.matmul(
                        ps[:, c0p:],
                        kT[et][:, jt * P:(jt + 1) * P],
                        qT[et][:, ic * 512 + c0p:(ic + 1) * 512],
                        start=(et == 0), stop=(et == DT - 1),
                    )
                st = stp.tile([P, 512], f32r, tag="st", name="st")
                psum_copy(st[:, c0:], ps[:, c0:], relu=True)
                if jt >= 4 * ic:
                    nc.gpsimd.tensor_tensor(
                        out=st[:, c0:c0 + P], in0=st[:, c0:c0 + P],
                        in1=tri[:], op=OP.mult)
                STl.append(st)

            def emit_av(ib, STl, sum_g, ib_l):
                ps = ps_slot("ps_av")
                for jt in range(ib + 1):
                    nc.tensor.matmul(
                        ps[:, :D],
                        STl[jt][:, ib_l * P:(ib_l + 1) * P],
                        Vsl(jt),
                        start=(jt == 0), stop=(jt == ib),
                    )
                nc.vector.scalar_tensor_tensor(
                    out=H[ib][:], in0=ps[:, :D],
                    scalar=invpos[:, ib:ib + 1], in1=H[ib][:],
                    op0=OP.mult, op1=OP.add,
                    accum_out=sum_g[:, ib_l:ib_l + 1])

            def emit_h1(cp, l):
                for et in range(DT):
                    ps = ps_slot("ps_h1")
                    for dt_ in range(DT):
                        nc.tensor.matmul(
                            ps[:],
                            W["w1", l, dt_][:, et * P:(et + 1) * P],
                            HT[0][dt_][:, cp * 512:(cp + 1) * 512],
                            start=(dt_ == 0), stop=(dt_ == DT - 1),
                            skip_group_check=True,
                        )
                    dst = h1T[et][:, cp * 512:(cp + 1) * 512]
                    _alt[0] ^= 1
                    if use_b1:
                        nc.scalar.activation(dst, ps[:], AF.Relu,
                                             bias=b1_t[l][:, et:et + 1])
                    elif _alt[0]:
                        nc.scalar.activation(dst, ps[:], AF.Relu)
                    else:
                        nc.vector.tensor_scalar_max(dst, ps[:], 0.0)

            def emit_m2(nt, l, sum_g, k):
                ps = ps_slot("ps_m2")
                for et in range(DT):
                    nc.tensor.matmul(
                        ps[:, :D],
                        h1T[et][:, nt * P:(nt + 1) * P],
                        W["w2", l, et][:],
                        start=(et == 0), stop=(et == DT - 1),
                    )
                if use_b2:
                    nc.vector.scalar_tensor_tensor(
                        out=H[nt][:], in0=ps[:, :D], scalar=1.0,
                        in1=H[nt][:], op0=OP.mult, op1=OP.add)
                    nc.vector.tensor_tensor(out=H[nt][:], in0=H[nt][:],
                                            in1=b2_t[l][:], op=OP.add)
                    nc.vector.tensor_reduce(
                        out=sum_g[:, k:k + 1], in_=H[nt][:],
                        axis=mybir.AxisListType.X, op=OP.add)
                else:
                    nc.vector.scalar_tensor_tensor(
                        out=H[nt][:], in0=ps[:, :D], scalar=1.0,
                        in1=H[nt][:], op0=OP.mult, op1=OP.add,
                        accum_out=sum_g[:, k:k + 1])

            # ---------------- main layer loop ----------------
            for li in range(L * REPEAT):
                l = li % L

                sum1 = smallp.tile([P, NT], f32, tag="sum1", name="sum1")
                ssq1 = smallp.tile([P, NT], f32, tag="ssq1", name="ssq1")
                sum2 = smallp.tile([P, NT], f32, tag="sum2", name="sum2")
                ssq2 = smallp.tile([P, NT], f32, tag="ssq2", name="ssq2")

                def emit_s_chunk(ic):
                    STl = []
                    for jt in range(4 * ic + 4):
                        emit_s_block(ic, jt, STl)
                    return STl

                def emit_av_ln(ic, STl):
                    for ib_l in range(4):
                        ib = 4 * ic + ib_l
                        emit_av(ib, STl, sum1[:, 4 * ic:4 * ic + 4], ib_l)
                        ln_sumsq(H[ib], ssq1[:, ib:ib + 1], ib_l)
                    rstd, nmu = ln_group_stats(
                        sum1[:, 4 * ic:4 * ic + 4], ssq1[:, 4 * ic:4 * ic + 4],
                        4, "1")
                    for ib_l in range(4):
                        ib = 4 * ic + ib_l
                        ln_apply(H[ib], rstd, nmu, ib_l,
                                 ln1g_t[l] if use_ln1_gb else None,
                                 ln1b_t[l] if use_ln1_gb else None)

                def mlp_chunk(g):
                    for k in range(4):
                        nt = 4 * g + k
                        emit_m2(nt, l, sum2[:, 4 * g:4 * g + 4], k)
                        ln_sumsq(H[nt], ssq2[:, nt:nt + 1], k)
                    rstd, nmu = ln_group_stats(
                        sum2[:, 4 * g:4 * g + 4], ssq2[:, 4 * g:4 * g + 4],
                        4, "2")
                    for k in range(4):
                        nt = 4 * g + k
                        ln_apply(H[nt], rstd, nmu, k,
                                 ln2g_t[l] if use_ln2_gb else None,
                                 ln2b_t[l] if use_ln2_gb else None)
                        if li == L * REPEAT - 1:
                            nc.sync.dma_start(
                                out_d[nt * P:(nt + 1) * P, :], H[nt][:])

                # --- phase A: transposes + QKV interleaved with S/AV ---
                # pair-granular start: qk(cp0) halves consume transpose pairs
                # as soon as their two LN applies land
                emit_T_pair(0, HT)
                emit_qk_half(0, "wq", qT, l)
                emit_T_pair(1, HT)
                emit_qk_half(0, "wk", kT, l)
                emit_qk_half(1, "wq", qT, l)
                emit_qk_half(1, "wk", kT, l)
                emit_v(0, l)
                emit_T(1, HT)
                emit_v(1, l)
                st0 = emit_s_chunk(0)
                emit_T(2, HT)
                emit_qk(1, "wq", qT, l)
                emit_qk(1, "wk", kT, l)
                emit_T(3, HT)
                emit_av_ln(0, st0)
                emit_v(2, l)
                emit_v(3, l)
                st1 = emit_s_chunk(1)
                emit_qk(2, "wq", qT, l)
                emit_qk(2, "wk", kT, l)
                emit_av_ln(1, st1)
                emit_v(4, l)
                emit_v(5, l)
                st2 = emit_s_chunk(2)
                emit_qk(3, "wq", qT, l)
                emit_qk(3, "wk", kT, l)
                emit_v(6, l)
                emit_v(7, l)
                emit_av_ln(2, st2)
                # --- phase B: MLP (T2/h1 interleave into the attn tail) ---
                st3 = emit_s_chunk(3)
                emit_T(0, HT)
                emit_av_ln(3, st3)
                emit_h1(0, l)
                emit_T(1, HT)
                emit_h1(1, l)
                emit_T(2, HT)
                mlp_chunk(0)
                emit_h1(2, l)
                emit_T(3, HT)
                mlp_chunk(1)
                emit_h1(3, l)
                mlp_chunk(2)
                mlp_chunk(3)

    nc.finalize()
    return nc


def kernel(**inputs):
    global LAST_EXEC_NS, LAST_RESULTS
    import ml_dtypes
    from concourse import bass_utils

    bfloat16 = ml_dtypes.bfloat16

    x = np.asarray(inputs["x"], dtype=np.float32)
    wpe = np.asarray(inputs["wpe"], dtype=np.float32)
    assert x.shape == (B, N, D), x.shape

    use_b1 = bool(np.any(np.asarray(inputs["mlp_b1"]) != 0))
    use_b2 = bool(np.any(np.asarray(inputs["mlp_b2"]) != 0))
    use_ln1 = not (np.all(np.asarray(inputs["ln1_g"]) == 1)
                   and np.all(np.asarray(inputs["ln1_b"]) == 0))
    use_ln2 = not (np.all(np.asarray(inputs["ln2_g"]) == 1)
                   and np.all(np.asarray(inputs["ln2_b"]) == 0))

    key = (use_b1, use_b2, use_ln1, use_ln2)
    if key not in _CACHE:
        _CACHE[key] = _build_program(*key)
    nc = _CACHE[key]

    h0 = x + wpe[None, :, :]  # positional embedding folded in on host

    tri = np.tril(np.ones((P, P), dtype=np.float32)).T  # tri[j,i] = j<=i
    ident = np.eye(P, dtype=np.float32)
    pos = np.arange(N, dtype=np.float32).reshape(NT, P).T  # [P, NT]
    invpos = (1.0 / (pos + 1.0)).astype(np.float32)

    shared = {
        "wq": np.ascontiguousarray(inputs["Wq"]).astype(bfloat16),
        "wk": np.ascontiguousarray(inputs["Wk"]).astype(bfloat16),
        "wv": np.ascontiguousarray(inputs["Wv"]).astype(bfloat16),
        "w1": np.ascontiguousarray(inputs["mlp_W1"]).astype(bfloat16),
        "w2": np.ascontiguousarray(inputs["mlp_W2"], dtype=np.float32),
        "tri": tri, "ident": ident.astype(bfloat16), "invpos": invpos,
    }
    if use_b1:
        shared["b1"] = np.asarray(inputs["mlp_b1"], dtype=np.float32)
    if use_b2:
        shared["b2"] = np.asarray(inputs["mlp_b2"], dtype=np.float32)
    if use_ln1:
        shared["ln1g"] = np.asarray(inputs["ln1_g"], dtype=np.float32)
        shared["ln1b"] = np.asarray(inputs["ln1_b"], dtype=np.float32)
    if use_ln2:
        shared["ln2g"] = np.asarray(inputs["ln2_g"], dtype=np.float32)
        shared["ln2b"] = np.asarray(inputs["ln2_b"], dtype=np.float32)

    in_maps = [dict(shared, h0=np.ascontiguousarray(h0[c]).astype(bfloat16))
               for c in range(B)]
    global _last_in_maps
    _last_in_maps = in_maps

    res = bass_utils.run_bass_kernel_spmd(
        nc, in_maps, core_ids=list(range(B)), trace=PROFILE)
    LAST_EXEC_NS = res.exec_time_ns
    LAST_RESULTS = res
    return np.stack(
        [np.asarray(res.results[c]["out"]).astype(np.float32)
         for c in range(B)], axis=0)
